# revision 3
# baseline (speedup 1.0000x reference)
"""Multi-head attention (B=2, N=4096, D=768, H=12) on 8 NeuronCores.

Sharding: core c -> (batch b = c//4, head-group hg = c%4 of 3 heads).
Each core computes Q/K/V projections for its 3 heads from the transposed
input xT (bf16), per-head scores^T = K @ Q^T with keys on partitions,
softmax (exp on ScalarE, denominator via a ones-column folded into the
AV matmul), AV, and the output projection restricted to its heads' rows
of Wo, producing a [4096, 768] fp32 partial. The host sums the four
head-group partials per batch and adds bo (the row-parallel all-reduce
done at unshard time).
"""

import numpy as np
import ml_dtypes

DIM = 768
NUM_HEADS = 12
HEAD_DIM = 64
SCALE = HEAD_DIM ** -0.5
B = 2
N = 4096
N_CORES = 8
HG = 3               # heads per core
HD3 = HG * HEAD_DIM  # 192
BF16 = ml_dtypes.bfloat16

_cache = {}


def _build_program():
    import concourse.bass as bass
    import concourse.mybir as mybir
    import concourse.tile as tile
    from concourse import bacc

    fp32 = mybir.dt.float32
    bf16 = mybir.dt.bfloat16
    AF = mybir.ActivationFunctionType

    nc = bacc.Bacc("TRN2", target_bir_lowering=False, debug=False,
                   num_devices=N_CORES)

    xt_d = nc.dram_tensor("xt", [DIM, N], bf16, kind="ExternalInput")
    wq_d = nc.dram_tensor("wq", [DIM, HD3], bf16, kind="ExternalInput")
    wk_d = nc.dram_tensor("wk", [DIM, HD3], bf16, kind="ExternalInput")
    wv_d = nc.dram_tensor("wv", [DIM, HG * 65], bf16, kind="ExternalInput")
    wo_d = nc.dram_tensor("wo", [HD3, DIM], bf16, kind="ExternalInput")
    bq_d = nc.dram_tensor("bq", [1, HD3], bf16, kind="ExternalInput")
    bk_d = nc.dram_tensor("bk", [1, HD3], bf16, kind="ExternalInput")
    bv_d = nc.dram_tensor("bv", [1, HG * 65], bf16, kind="ExternalInput")
    out_d = nc.dram_tensor("out", [N, DIM], fp32, kind="ExternalOutput")

    KC = DIM // 128      # 6 contraction chunks
    NQB = N // 512       # 8 query blocks of 512
    NKB = N // 128       # 32 key blocks of 128
    G = 2                # key blocks per exp group
    NSEQ = N // 128      # 32 output row blocks
    V_W = HG * 65        # 195: v columns incl. per-head ones column

    with tile.TileContext(nc) as tc:
        with (
            tc.tile_pool(name="const", bufs=1) as cpool,
            tc.tile_pool(name="big", bufs=1) as bpool,
            tc.tile_pool(name="work", bufs=3) as wpool,
            tc.tile_pool(name="psum", bufs=2, space="PSUM") as ppool,
        ):
            # ---- load inputs -------------------------------------------------
            xt = []
            for k in range(KC):
                t = cpool.tile([128, N], bf16, tag=f"xt{k}")
                nc.sync.dma_start(t[:], xt_d.ap()[k * 128:(k + 1) * 128, :])
                xt.append(t)
            wq, wk, wv = [], [], []
            for k in range(KC):
                t = cpool.tile([128, HD3], bf16, tag=f"wq{k}")
                nc.sync.dma_start(t[:], wq_d.ap()[k * 128:(k + 1) * 128, :])
                wq.append(t)
                t = cpool.tile([128, HD3], bf16, tag=f"wk{k}")
                nc.sync.dma_start(t[:], wk_d.ap()[k * 128:(k + 1) * 128, :])
                wk.append(t)
                t = cpool.tile([128, V_W], bf16, tag=f"wv{k}")
                nc.sync.dma_start(t[:], wv_d.ap()[k * 128:(k + 1) * 128, :])
                wv.append(t)
            wo = []
            for h in range(HG):
                t = cpool.tile([64, DIM], bf16, tag=f"wo{h}")
                nc.sync.dma_start(t[:], wo_d.ap()[h * 64:(h + 1) * 64, :])
                wo.append(t)
            bq = cpool.tile([1, HD3], bf16, tag="bq")
            nc.sync.dma_start(bq[:], bq_d.ap()[:])
            bk = cpool.tile([1, HD3], bf16, tag="bk")
            nc.sync.dma_start(bk[:], bk_d.ap()[:])
            bv = cpool.tile([1, V_W], bf16, tag="bv")
            nc.sync.dma_start(bv[:], bv_d.ap()[:])
            ones = cpool.tile([1, 512], bf16, tag="ones")
            nc.gpsimd.memset(ones[:], 1.0)

            # ---- K^T and Q^T projections: [64, 4096] per head ---------------
            kt = [bpool.tile([64, N], bf16, tag=f"kt{h}", name=f"kt{h}") for h in range(HG)]
            qt = [bpool.tile([64, N], bf16, tag=f"qt{h}", name=f"qt{h}") for h in range(HG)]
            for h in range(HG):
                for nb in range(NQB):
                    csl = slice(nb * 512, (nb + 1) * 512)
                    for dst, w, bias in ((kt, wk, bk), (qt, wq, bq)):
                        ps = ppool.tile([64, 512], fp32, tag="proj")
                        for k in range(KC):
                            nc.tensor.matmul(
                                ps[:], w[k][:, h * 64:(h + 1) * 64],
                                xt[k][:, csl],
                                start=(k == 0), stop=False)
                        nc.tensor.matmul(
                            ps[:], bias[:, h * 64:(h + 1) * 64], ones[:],
                            start=False, stop=True)
                        nc.vector.tensor_copy(dst[h][:, csl], ps[:])

            # ---- V projection: [128, 195] per seq block ---------------------
            # layout: v_sb[:, s*195 + h*65 : +64] = V rows, col h*65+64 = 1.0
            v_sb = bpool.tile([128, NKB * V_W], bf16, tag="v")
            for s in range(NKB):
                ssl = slice(s * 128, (s + 1) * 128)
                ps = ppool.tile([128, V_W], fp32, tag="proj")
                for k in range(KC):
                    nc.tensor.matmul(ps[:], xt[k][:, ssl], wv[k][:],
                                     start=(k == 0), stop=False)
                nc.tensor.matmul(ps[:], ones[:, 0:128], bv[:],
                                 start=False, stop=True)
                nc.vector.tensor_copy(
                    v_sb[:, s * V_W:(s + 1) * V_W], ps[:])

            # ---- attention ---------------------------------------------------
            attn = [bpool.tile([64, N], bf16, tag=f"attn{h}", name=f"attn{h}") for h in range(HG)]
            for h in range(HG):
                for qb in range(NQB):
                    qsl = slice(qb * 512, (qb + 1) * 512)
                    av = ppool.tile([65, 512], fp32, tag="av")
                    for g in range(NKB // G):
                        sc = ppool.tile([128, G * 512], fp32, tag="scores")
                        for j in range(G):
                            kb = g * G + j
                            nc.tensor.matmul(
                                sc[:, j * 512:(j + 1) * 512],
                                kt[h][:, kb * 128:(kb + 1) * 128],
                                qt[h][:, qsl],
                                start=True, stop=True)
                        probs = wpool.tile([128, G * 512], bf16, tag="probs")
                        nc.scalar.activation(probs[:], sc[:], AF.Exp)
                        for j in range(G):
                            kb = g * G + j
                            nc.tensor.matmul(
                                av[:],
                                v_sb[:, kb * V_W + h * 65:kb * V_W + (h + 1) * 65],
                                probs[:, j * 512:(j + 1) * 512],
                                start=(kb == 0), stop=(kb == NKB - 1))
                    r_row = wpool.tile([1, 512], fp32, tag="r_row")
                    nc.vector.reciprocal(r_row[:], av[64:65, :])
                    r_bc = wpool.tile([64, 512], fp32, tag="r_bc")
                    nc.gpsimd.partition_broadcast(r_bc[:], r_row[:])
                    nc.vector.tensor_mul(attn[h][:, qsl], av[0:64, :], r_bc[:])

            # ---- output projection: out[s*128:+128, :] partial --------------
            for s in range(NSEQ):
                ssl = slice(s * 128, (s + 1) * 128)
                ob = wpool.tile([128, DIM], fp32, tag="out_sb")
                for n2 in range(2):
                    nsl = slice(n2 * 384, (n2 + 1) * 384)
                    ps = ppool.tile([128, 384], fp32, tag="proj")
                    for h in range(HG):
                        nc.tensor.matmul(ps[:], attn[h][:, ssl],
                                         wo[h][:, nsl],
                                         start=(h == 0), stop=(h == HG - 1))
                    nc.vector.tensor_copy(ob[:, nsl], ps[:])
                nc.sync.dma_start(out_d.ap()[ssl, :], ob[:])

    nc.compile()
    return nc


def _get_program():
    if "nc" not in _cache:
        _cache["nc"] = _build_program()
    return _cache["nc"]


def kernel(x, Wq, bq, Wk, bk, Wv, bv, Wo, bo):
    from concourse import bass_utils

    x = np.asarray(x, np.float32)
    Wq = np.asarray(Wq, np.float32); bq = np.asarray(bq, np.float32)
    Wk = np.asarray(Wk, np.float32); bk = np.asarray(bk, np.float32)
    Wv = np.asarray(Wv, np.float32); bv = np.asarray(bv, np.float32)
    Wo = np.asarray(Wo, np.float32); bo = np.asarray(bo, np.float32)

    nc = _get_program()

    in_maps = []
    for c in range(N_CORES):
        b, hg = divmod(c, 4)
        sl = slice(HD3 * hg, HD3 * (hg + 1))
        wv_ext = np.zeros((DIM, HG * 65), np.float32)
        bv_ext = np.zeros((1, HG * 65), np.float32)
        for h in range(HG):
            wv_ext[:, h * 65:h * 65 + 64] = Wv[:, HD3 * hg + h * 64:HD3 * hg + (h + 1) * 64]
            bv_ext[0, h * 65:h * 65 + 64] = bv[HD3 * hg + h * 64:HD3 * hg + (h + 1) * 64]
            bv_ext[0, h * 65 + 64] = 1.0
        in_maps.append({
            "xt": np.ascontiguousarray(x[b].T).astype(BF16),
            "wq": (Wq[:, sl] * SCALE).astype(BF16),
            "wk": Wk[:, sl].astype(BF16),
            "wv": wv_ext.astype(BF16),
            "wo": Wo[sl, :].astype(BF16),
            "bq": (bq[None, sl] * SCALE).astype(BF16),
            "bk": bk[None, sl].astype(BF16),
            "bv": bv_ext.astype(BF16),
        })

    _cache["in_maps"] = in_maps
    res = bass_utils.run_bass_kernel_spmd(nc, in_maps, core_ids=list(range(N_CORES)))
    _cache["last_results"] = res

    out = np.zeros((B, N, DIM), np.float32)
    for c in range(N_CORES):
        b = c // 4
        out[b] += res.results[c]["out"]
    out += bo[None, None, :]
    return out


# revision 5
# speedup vs baseline: 1.4241x; 1.4241x over previous
"""Multi-head attention (B=2, N=4096, D=768, H=12) on 8 NeuronCores.

Sharding: core c -> (batch b = c//4, head-group hg = c%4 of 3 heads).
Each core computes Q/K/V projections for its 3 heads from the transposed
input xT (bf16), per-head scores^T = K @ Q^T with keys on partitions,
softmax (exp on ScalarE, denominator via a ones-column folded into the
AV matmul), AV, and the output projection restricted to its heads' rows
of Wo, producing a [4096, 768] fp32 partial. The host sums the four
head-group partials per batch and adds bo (the row-parallel all-reduce
done at unshard time).

Heads 0+1 are packed into combined [128, N] K^T/Q^T tiles so their score
matmuls run concurrently in disjoint PE row groups; head 2 and the
output projection are zero-padded to a full 128-row contraction. This
keeps the PE HAM clock gate at 8/8 (half-array matmuls otherwise leave
the array looking idle and the clock drops to 1.2 GHz).
"""

import numpy as np
import ml_dtypes

DIM = 768
NUM_HEADS = 12
HEAD_DIM = 64
SCALE = HEAD_DIM ** -0.5
B = 2
N = 4096
N_CORES = 8
HG = 3               # heads per core
HD3 = HG * HEAD_DIM  # 192
BF16 = ml_dtypes.bfloat16

_cache = {}


def _build_program():
    import concourse.mybir as mybir
    import concourse.tile as tile
    from concourse import bacc

    fp32 = mybir.dt.float32
    bf16 = mybir.dt.bfloat16
    AF = mybir.ActivationFunctionType

    nc = bacc.Bacc("TRN2", target_bir_lowering=False, debug=False,
                   num_devices=N_CORES)

    xt_d = nc.dram_tensor("xt", [DIM, N], bf16, kind="ExternalInput")
    wq_d = nc.dram_tensor("wq", [DIM, HD3], bf16, kind="ExternalInput")
    wk_d = nc.dram_tensor("wk", [DIM, HD3], bf16, kind="ExternalInput")
    wv_d = nc.dram_tensor("wv", [DIM, HG * 65], bf16, kind="ExternalInput")
    wo_d = nc.dram_tensor("wo", [HD3, DIM], bf16, kind="ExternalInput")
    bq_d = nc.dram_tensor("bq", [1, HD3], bf16, kind="ExternalInput")
    bk_d = nc.dram_tensor("bk", [1, HD3], bf16, kind="ExternalInput")
    bv_d = nc.dram_tensor("bv", [1, HG * 65], bf16, kind="ExternalInput")
    out_d = nc.dram_tensor("out", [N, DIM], fp32, kind="ExternalOutput")

    KC = DIM // 128      # 6 contraction chunks
    NQB = N // 512       # 8 query blocks of 512
    NKB = N // 128       # 32 key blocks of 128
    NSEQ = N // 128      # 32 output row blocks
    V_W = HG * 65        # 195: v columns incl. per-head ones column

    with tile.TileContext(nc) as tc:
        with (
            tc.tile_pool(name="const", bufs=1) as cpool,
            tc.tile_pool(name="big", bufs=1) as bpool,
            tc.tile_pool(name="work", bufs=4) as wpool,
            tc.tile_pool(name="psum", bufs=2, space="PSUM") as ppool,
        ):
            # ---- load inputs (small weights first: first matmuls need them)
            wq, wk, wv = [], [], []
            for k in range(KC):
                t = cpool.tile([128, HD3], bf16, tag=f"wq{k}")
                nc.sync.dma_start(t[:], wq_d.ap()[k * 128:(k + 1) * 128, :])
                wq.append(t)
                t = cpool.tile([128, HD3], bf16, tag=f"wk{k}")
                nc.sync.dma_start(t[:], wk_d.ap()[k * 128:(k + 1) * 128, :])
                wk.append(t)
                t = cpool.tile([128, V_W], bf16, tag=f"wv{k}")
                nc.sync.dma_start(t[:], wv_d.ap()[k * 128:(k + 1) * 128, :])
                wv.append(t)
            # wo padded to 128 rows (rows 64.. are zeros) for cdim-128 matmuls
            wo = []
            for h in range(HG):
                t = cpool.tile([128, DIM], bf16, tag=f"wo{h}", name=f"wo{h}")
                nc.sync.dma_start(t[0:64, :], wo_d.ap()[h * 64:(h + 1) * 64, :])
                nc.gpsimd.memset(t[64:128, :], 0.0)
                wo.append(t)
            bq = cpool.tile([1, HD3], bf16, tag="bq")
            nc.sync.dma_start(bq[:], bq_d.ap()[:])
            bk = cpool.tile([1, HD3], bf16, tag="bk")
            nc.sync.dma_start(bk[:], bk_d.ap()[:])
            bv = cpool.tile([1, V_W], bf16, tag="bv")
            nc.sync.dma_start(bv[:], bv_d.ap()[:])
            ones = cpool.tile([1, 512], bf16, tag="ones")
            nc.gpsimd.memset(ones[:], 1.0)
            xt = []
            for k in range(KC):
                t = cpool.tile([128, N], bf16, tag=f"xt{k}")
                nc.sync.dma_start(t[:], xt_d.ap()[k * 128:(k + 1) * 128, :])
                xt.append(t)

            # ---- K^T / Q^T: heads 0,1 combined in [128, N]; head 2 padded --
            kt01 = bpool.tile([128, N], bf16, tag="kt01")
            qt01 = bpool.tile([128, N], bf16, tag="qt01")
            kt2 = bpool.tile([128, N], bf16, tag="kt2")
            qt2 = bpool.tile([128, N], bf16, tag="qt2")
            nc.gpsimd.memset(kt2[64:128, :], 0.0)
            nc.gpsimd.memset(qt2[64:128, :], 0.0)
            for nb in range(NQB):
                csl = slice(nb * 512, (nb + 1) * 512)
                for dst01, dst2, w, bias in ((kt01, kt2, wk, bk),
                                             (qt01, qt2, wq, bq)):
                    ps = ppool.tile([128, 512], fp32, tag="proj")
                    for k in range(KC):
                        nc.tensor.matmul(ps[:], w[k][:, 0:128], xt[k][:, csl],
                                         start=(k == 0), stop=False)
                    nc.tensor.matmul(ps[:], bias[:, 0:128], ones[:],
                                     start=False, stop=True)
                    nc.vector.tensor_copy(dst01[:, csl], ps[:])
                    ps = ppool.tile([64, 512], fp32, tag="proj")
                    for k in range(KC):
                        nc.tensor.matmul(ps[:], w[k][:, 128:192], xt[k][:, csl],
                                         start=(k == 0), stop=False)
                    nc.tensor.matmul(ps[:], bias[:, 128:192], ones[:],
                                     start=False, stop=True)
                    nc.vector.tensor_copy(dst2[0:64, csl], ps[:])

            # ---- V: [128, 195] per seq block; col h*65+64 is the ones col --
            v_sb = bpool.tile([128, NKB * V_W], bf16, tag="v")
            for s in range(NKB):
                ssl = slice(s * 128, (s + 1) * 128)
                ps = ppool.tile([128, V_W], fp32, tag="proj")
                for k in range(KC):
                    nc.tensor.matmul(ps[:], xt[k][:, ssl], wv[k][:],
                                     start=(k == 0), stop=False)
                nc.tensor.matmul(ps[:], ones[:, 0:128], bv[:],
                                 start=False, stop=True)
                nc.vector.tensor_copy(v_sb[:, s * V_W:(s + 1) * V_W], ps[:])

            # ---- attention; attn tiles padded to 128 rows (zeros above) ----
            attn = []
            for h in range(HG):
                t = bpool.tile([128, N], bf16, tag=f"attn{h}", name=f"attn{h}")
                nc.gpsimd.memset(t[64:128, :], 0.0)
                attn.append(t)

            def normalize(av, h, qsl):
                # decouple from PSUM quickly, then recip/bcast/mul from SBUF
                av_sb = wpool.tile([65, 512], fp32, tag="av_sb")
                nc.vector.tensor_copy(av_sb[:], av[:])
                r_row = wpool.tile([1, 512], fp32, tag="r_row")
                nc.vector.reciprocal(r_row[:], av_sb[64:65, :])
                r_bc = wpool.tile([64, 512], fp32, tag="r_bc")
                nc.gpsimd.partition_broadcast(r_bc[:], r_row[:])
                nc.vector.tensor_mul(attn[h][0:64, qsl], av_sb[0:64, :], r_bc[:])

            for qb in range(NQB):
                qsl = slice(qb * 512, (qb + 1) * 512)
                # heads 0,1: packed scores (disjoint PE row groups)
                av0 = ppool.tile([65, 512], fp32, tag="av")
                av1 = ppool.tile([65, 512], fp32, tag="av")
                for kb in range(NKB):
                    ksl = slice(kb * 128, (kb + 1) * 128)
                    sc = ppool.tile([128, 1024], fp32, tag="scores")
                    nc.tensor.matmul(sc[:, 0:512], kt01[0:64, ksl],
                                     qt01[0:64, qsl], start=True, stop=True)
                    nc.tensor.matmul(sc[:, 512:1024], kt01[64:128, ksl],
                                     qt01[64:128, qsl], start=True, stop=True)
                    probs = wpool.tile([128, 1024], bf16, tag="probs")
                    nc.scalar.activation(probs[:], sc[:], AF.Exp)
                    nc.tensor.matmul(
                        av0[:], v_sb[:, kb * V_W + 0 * 65:kb * V_W + 0 * 65 + 65],
                        probs[:, 0:512],
                        start=(kb == 0), stop=(kb == NKB - 1))
                    nc.tensor.matmul(
                        av1[:], v_sb[:, kb * V_W + 1 * 65:kb * V_W + 1 * 65 + 65],
                        probs[:, 512:1024],
                        start=(kb == 0), stop=(kb == NKB - 1))
                normalize(av0, 0, qsl)
                normalize(av1, 1, qsl)
                # head 2: zero-padded cdim-128 scores, two key blocks per exp
                av2 = ppool.tile([65, 512], fp32, tag="av")
                for g in range(NKB // 2):
                    sc = ppool.tile([128, 1024], fp32, tag="scores")
                    for j in range(2):
                        kb = 2 * g + j
                        nc.tensor.matmul(
                            sc[:, j * 512:(j + 1) * 512],
                            kt2[:, kb * 128:(kb + 1) * 128], qt2[:, qsl],
                            start=True, stop=True)
                    probs = wpool.tile([128, 1024], bf16, tag="probs")
                    nc.scalar.activation(probs[:], sc[:], AF.Exp)
                    for j in range(2):
                        kb = 2 * g + j
                        nc.tensor.matmul(
                            av2[:], v_sb[:, kb * V_W + 2 * 65:kb * V_W + 2 * 65 + 65],
                            probs[:, j * 512:(j + 1) * 512],
                            start=(kb == 0), stop=(kb == NKB - 1))
                normalize(av2, 2, qsl)

            # ---- output projection (cdim 128, zero-padded heads) -----------
            for s in range(NSEQ):
                ssl = slice(s * 128, (s + 1) * 128)
                ob = wpool.tile([128, DIM], fp32, tag="out_sb")
                for n2 in range(2):
                    nsl = slice(n2 * 384, (n2 + 1) * 384)
                    ps = ppool.tile([128, 384], fp32, tag="proj")
                    for h in range(HG):
                        nc.tensor.matmul(ps[:], attn[h][:, ssl], wo[h][:, nsl],
                                         start=(h == 0), stop=(h == HG - 1))
                    nc.vector.tensor_copy(ob[:, nsl], ps[:])
                nc.sync.dma_start(out_d.ap()[ssl, :], ob[:])

    nc.compile()
    return nc


def _get_program():
    if "nc" not in _cache:
        _cache["nc"] = _build_program()
    return _cache["nc"]


def _make_in_maps(x, Wq, bq, Wk, bk, Wv, bv, Wo):
    in_maps = []
    for c in range(N_CORES):
        b, hg = divmod(c, 4)
        sl = slice(HD3 * hg, HD3 * (hg + 1))
        wv_ext = np.zeros((DIM, HG * 65), np.float32)
        bv_ext = np.zeros((1, HG * 65), np.float32)
        for h in range(HG):
            wv_ext[:, h * 65:h * 65 + 64] = Wv[:, HD3 * hg + h * 64:HD3 * hg + (h + 1) * 64]
            bv_ext[0, h * 65:h * 65 + 64] = bv[HD3 * hg + h * 64:HD3 * hg + (h + 1) * 64]
            bv_ext[0, h * 65 + 64] = 1.0
        in_maps.append({
            "xt": np.ascontiguousarray(x[b].T).astype(BF16),
            "wq": (Wq[:, sl] * SCALE).astype(BF16),
            "wk": Wk[:, sl].astype(BF16),
            "wv": wv_ext.astype(BF16),
            "wo": Wo[sl, :].astype(BF16),
            "bq": (bq[None, sl] * SCALE).astype(BF16),
            "bk": bk[None, sl].astype(BF16),
            "bv": bv_ext.astype(BF16),
        })
    return in_maps


def kernel(x, Wq, bq, Wk, bk, Wv, bv, Wo, bo):
    from concourse import bass_utils

    x = np.asarray(x, np.float32)
    Wq = np.asarray(Wq, np.float32); bq = np.asarray(bq, np.float32)
    Wk = np.asarray(Wk, np.float32); bk = np.asarray(bk, np.float32)
    Wv = np.asarray(Wv, np.float32); bv = np.asarray(bv, np.float32)
    Wo = np.asarray(Wo, np.float32); bo = np.asarray(bo, np.float32)

    nc = _get_program()
    in_maps = _make_in_maps(x, Wq, bq, Wk, bk, Wv, bv, Wo)
    _cache["in_maps"] = in_maps
    res = bass_utils.run_bass_kernel_spmd(nc, in_maps, core_ids=list(range(N_CORES)))
    _cache["last_results"] = res

    out = np.zeros((B, N, DIM), np.float32)
    for c in range(N_CORES):
        out[c // 4] += res.results[c]["out"]
    out += bo[None, None, :]
    return out
